# revision 23
# baseline (speedup 1.0000x reference)
"""CapsuleLayer (dynamic routing) Trainium2 Bass kernel — v3.

Math (per example b):
  u_hat[b,i,o,n] = sum_v x[b,i,v] * W[i,o,v,n]        I=1152, O=10, V=8, N=16
  b_logits = 0; repeat n_routing times:
    c = softmax_o(b_logits); s = sum_i c*u_hat; out = squash(s)
    if not last: b_logits += sum_n u_hat*out

Distribution: batch B=256 sharded over 8 cores (32 each). W replicated.

v3 structure (vs v2):
  - (o,n)-inner free layout: U[p=(b,il), ib, (o,n)] so phase-1 psum->SBUF
    copies are contiguous ACT copies and s-matmul rhs slices are contiguous.
  - logit linearity: b_t = u_hat . (v_0+..+v_{t-1}), so each iteration's
    tree uses the running vsum and writes the logits directly (no bb
    accumulate/copy passes).
  - coarse 2-half agreement trees (mul + 4 halving adds per half) on DVE
    with in-place scratch reuse; fewer per-op overheads.
  - phase-1 build interleaved with it0 routing so DVE trees overlap the
    PE build stream; xe slices prefetched (bufs=3) on alternating queues.
  - squash rsqrt: DVE bit-hack + 1 Newton step.

Per-core layout (chunk = 8 examples, 4 chunks), i = ib*16 + il:
  K partitions k = il*8+v   (contraction rows of the u_hat matmul)
  M partitions p = b*16+il  (rows of u_hat / routing state)
  U[c]  [128, 72, 160] bf16   U[(b,il), ib, (o,n)]
  XBD   [128, 9, 128] bf16 x stationary eighth: XBD[(il,v), ibl, (b,il')]
  xt2   [128, 72, 32] bf16 dense x stationary: xt2[(il,v), ib, b]
  w2n   [128, 72, 160] bf16  w2n[(il,v), ib, (o,n)] = W[ib*16+il, o, v, n]
  CBD[c] [128, 80, 72] bf16 block-diag c stationary: CBD[(b,il),(o,b'),ib]
  u_hat matmul (per ib): psum[(b,il'),(o,n)] = XBD[:,ibl,:].T @ w2n[:,ib,:]
  s matmul (per it,c): psum[(o,b'),(o',n)] += CBD[:,:,ib].T @ U[:,ib,:]
    -> diagonal o==o' holds s[b', o, n]  (extracted via DVE shuffles)
"""

import os
import sys

import numpy as np

_TRN_REPO = "/opt/trn_rl_repo"
if _TRN_REPO not in sys.path:
    sys.path.insert(0, _TRN_REPO)

EPS = 1e-10
B, I, V, O, N = 256, 1152, 8, 10, 16
NCORES = 8
BLOC = B // NCORES          # 32 examples per core
BC = 8                      # examples per chunk
NCHUNK = BLOC // BC         # 4
IB = I // 16                # 72 i-blocks
ON = O * N                  # 160
EI = 8                      # xbd load slices per chunk
IBE = IB // EI              # 9 i-blocks per slice
HH = IB // 2                # 36 i-blocks per tree half
RSQRT_MAGIC = 0x5F3759DF
NEWTON = 1


def _squash(nc, mybir, small, sv, nsq_src):
    """Emit the squash scale chain, fully on DVE (tiny ops; avoids
    cross-engine ping-pong latency in the per-chunk chains).

    nsq_src: AP [sv, O, N] view (strided ok) of s to square-reduce.
    Returns sc AP [sv, O] (the per-(b,o) scale nsq*rsqrt(...)).
    """
    f32 = mybir.dt.float32
    i32 = mybir.dt.int32
    AX = mybir.AxisListType
    OP = mybir.AluOpType

    sq = small.tile([sv, O, N], f32, tag="sq", name="sq")
    nc.vector.tensor_mul(sq[:], nsq_src, nsq_src)
    nsq = small.tile([sv, O], f32, tag="nsq", name="nsq")
    nc.vector.tensor_reduce(nsq[:], sq[:], axis=AX.X, op=OP.add)
    np1 = small.tile([sv, O], f32, tag="np1", name="np1")
    nc.gpsimd.tensor_scalar_add(np1[:], nsq[:], 1.0)
    d1 = small.tile([sv, O], f32, tag="d1", name="d1")
    nc.gpsimd.tensor_mul(d1[:], np1[:], np1[:])
    nse = small.tile([sv, O], f32, tag="nse", name="nse")
    nc.gpsimd.tensor_scalar_add(nse[:], nsq[:], EPS)
    dd = small.tile([sv, O], f32, tag="dd", name="dd")
    nc.gpsimd.tensor_mul(dd[:], d1[:], nse[:])
    yy = small.tile([sv, O], f32, tag="yy", name="yy")
    nc.vector.tensor_scalar(
        yy[:].bitcast(i32), dd[:].bitcast(i32), 1, None,
        op0=OP.logical_shift_right,
    )
    nc.vector.tensor_scalar(
        yy[:].bitcast(i32), yy[:].bitcast(i32), -1, RSQRT_MAGIC,
        op0=OP.mult, op1=OP.add,
    )
    for _ in range(NEWTON):
        y2 = small.tile([sv, O], f32, tag="y2", name="y2")
        nc.gpsimd.tensor_mul(y2[:], yy[:], yy[:])
        t2 = small.tile([sv, O], f32, tag="t2", name="t2")
        nc.gpsimd.tensor_mul(t2[:], y2[:], dd[:])
        u2 = small.tile([sv, O], f32, tag="u2", name="u2")
        nc.gpsimd.tensor_scalar(
            u2[:], t2[:], -0.5, 1.5, op0=OP.mult, op1=OP.add
        )
        yn = small.tile([sv, O], f32, tag="yn", name="yn")
        nc.gpsimd.tensor_mul(yn[:], yy[:], u2[:])
        yy = yn
    sc = small.tile([sv, O], f32, tag="sc", name="sc")
    nc.vector.tensor_mul(sc[:], nsq[:], yy[:])
    return sc


def _build(n_routing: int, reps: int = 1):
    import concourse.bacc as bacc
    import concourse.tile as tile
    from concourse import mybir

    nc = bacc.Bacc("TRN2", target_bir_lowering=False, debug=False)
    f32 = mybir.dt.float32
    bf16 = mybir.dt.bfloat16

    xbdh = nc.dram_tensor(
        "xbdh", [NCHUNK, 128, IB, 128], bf16, kind="ExternalInput"
    )
    w2 = nc.dram_tensor("w2", [128, IB, ON], bf16, kind="ExternalInput")
    xt2 = nc.dram_tensor("xt2", [128, IB, BLOC], bf16, kind="ExternalInput")
    out_d = nc.dram_tensor("out", [BLOC, O, N], f32, kind="ExternalOutput")

    with tile.TileContext(nc) as tc:
        with (
            tc.tile_pool(name="state", bufs=1) as state,
            tc.tile_pool(name="small", bufs=2) as small,
            tc.tile_pool(name="tree", bufs=1) as tree,
            tc.tile_pool(name="xep", bufs=3) as xep,
            tc.tile_pool(name="psA", bufs=3, space="PSUM") as psA,
            tc.tile_pool(name="psR", bufs=1, space="PSUM") as psR,
            tc.tile_pool(name="psS", bufs=2, space="PSUM") as psS,
            tc.tile_pool(name="psW", bufs=1, space="PSUM") as psW,
        ):
            xt2s = state.tile([128, IB, BLOC], bf16)
            nc.gpsimd.dma_start(out=xt2s[:], in_=xt2[:])
            # w2 in quarters so it0 matmuls start as soon as data lands
            IBQ = IB // 4
            w2qs = []
            for k in range(4):
                w2q = state.tile([128, IBQ, ON], bf16, tag=f"w2q{k}")
                nc.sync.dma_start(
                    out=w2q[:], in_=w2[:, k * IBQ:(k + 1) * IBQ, :]
                )
                w2qs.append(w2q)

            def w2sl(ib):
                return w2qs[ib // IBQ][:, ib % IBQ, :]

            Us = [
                state.tile([128, IB, ON], bf16, tag=f"U{c}", name=f"U{c}")
                for c in range(NCHUNK)
            ]
            cbds = [
                state.tile([128, 80, IB], bf16, tag=f"cbd{c}", name=f"cbd{c}")
                for c in range(NCHUNK)
            ] if n_routing > 1 else []
            for cb in cbds:
                nc.gpsimd.memset(cb[:], 0.0)

            for rep in range(reps):
                _body(
                    nc, tc, mybir, tile, state, small, tree, xep,
                    psA, psR, psS, psW, xbdh, w2sl, xt2s, Us, cbds,
                    out_d, n_routing,
                )

    nc.compile()
    return nc


def _phase1(nc, mybir, xep, psA, xbdh, w2sl, Us, c):
    f32 = mybir.dt.float32
    bf16 = mybir.dt.bfloat16
    for e in range(EI):
        xe = xep.tile([128, IBE, 128], bf16, tag="xbd", name="xe")
        q = nc.gpsimd if e % 2 == 0 else nc.sync
        q.dma_start(out=xe[:], in_=xbdh[c, :, e * IBE:(e + 1) * IBE, :])
        for g in range(IBE // 3):
            ps = psA.tile([128, 3, ON], f32, tag="psA", name="psa")
            for j in range(3):
                ibl = g * 3 + j
                ib = e * IBE + ibl
                nc.tensor.matmul(
                    ps[:, j, :], xe[:, ibl, :], w2sl(ib),
                    start=True, stop=True,
                )
            ib0 = e * IBE + g * 3
            # contiguous flattened [128, 480] f32 -> bf16 copy on ACT
            dst = Us[c][:, ib0:ib0 + 3, :].rearrange("p a b -> p (a b)")
            src = ps[:].rearrange("p a b -> p (a b)")
            nc.scalar.copy(dst, src)


def _body(nc, tc, mybir, tile, state, small, tree, xep, psA, psR, psS, psW,
          xbdh, w2sl, xt2s, Us, cbds, out_d, n_routing):
    f32 = mybir.dt.float32
    bf16 = mybir.dt.bfloat16
    AX = mybir.AxisListType
    OP = mybir.AluOpType
    AF = mybir.ActivationFunctionType

    # ---------------- iteration-0 s shortcut: r = sum_i u_hat ------------
    # psR[b, (o,n)] = sum_ib xt2s[:, ib, :].T @ w2n[:, ib, :]
    pr = psR.tile([BLOC, ON], f32, tag="psR", name="pr")
    for ib in range(IB):
        nc.tensor.matmul(
            pr[:], xt2s[:, ib, :], w2sl(ib),
            start=(ib == 0), stop=(ib == IB - 1),
        )
    s0 = small.tile([BLOC, ON], f32, tag="s0", name="s0")
    nc.scalar.mul(s0[:], pr[:], 1.0 / O)
    # squash over n for all 32 examples; s0 layout (o, n)
    s0v = s0[:].rearrange("p (o n) -> p o n", n=N)
    sc0 = _squash(nc, mybir, small, BLOC, s0v)
    v3b0 = small.tile([BLOC, ON], bf16, tag="v3b0", name="v3b0")
    nc.vector.tensor_mul(
        v3b0[:].rearrange("p (o n) -> p o n", n=N),
        s0v,
        sc0.unsqueeze(2).broadcast_to([BLOC, O, N]),
    )
    vrep0s = []
    for c in range(NCHUNK):
        vrep = small.tile(
            [128, ON], bf16, tag=f"vrep0_{c}", name=f"vr0_{c}", bufs=1
        )
        for q in range(4):
            nc.vector.stream_shuffle(
                vrep[q * 32:(q + 1) * 32, :],
                v3b0[:],
                [c * BC + 2 * q + (j // 16) for j in range(32)],
            )
        vrep0s.append(vrep)

    # ---------------- phase 1 (u_hat) interleaved with it0 routing -------
    vsums = [None] * NCHUNK
    _phase1(nc, mybir, xep, psA, xbdh, w2sl, Us, 0)
    _phase1(nc, mybir, xep, psA, xbdh, w2sl, Us, 1)
    _chain(nc, tc, mybir, small, tree, psW, Us[0],
           cbds[0] if cbds else None, vrep0s, vsums, out_d, None,
           0, 0, n_routing)
    _phase1(nc, mybir, xep, psA, xbdh, w2sl, Us, 2)
    _chain(nc, tc, mybir, small, tree, psW, Us[1],
           cbds[1] if cbds else None, vrep0s, vsums, out_d, None,
           1, 0, n_routing)
    _phase1(nc, mybir, xep, psA, xbdh, w2sl, Us, 3)
    _chain(nc, tc, mybir, small, tree, psW, Us[2],
           cbds[2] if cbds else None, vrep0s, vsums, out_d, None,
           2, 0, n_routing)
    _chain(nc, tc, mybir, small, tree, psW, Us[3],
           cbds[3] if cbds else None, vrep0s, vsums, out_d, None,
           3, 0, n_routing)

    # ---------------- its >= 1, software-pipelined emission --------------
    # Emit smm(next) before chain(cur) so each engine's FIFO alternates
    # chunks: PE streams s-matmuls back-to-back while DVE runs the
    # previous chunk's tree/softmax chain.
    seq = [(it, c) for it in range(1, n_routing) for c in range(NCHUNK)]
    pending = None
    for it, c in seq:
        sY = _smm(nc, mybir, small, psS, Us[c], cbds[c], c, it)
        if pending is not None:
            _chain(nc, tc, mybir, small, tree, psW, *pending)
        pending = (Us[c], cbds[c], vrep0s, vsums, out_d, sY, c, it,
                   n_routing)
    _chain(nc, tc, mybir, small, tree, psW, *pending)


def _smm(nc, mybir, small, psS, U, cbd, c, it):
    """s matmul for (it, c): accumulate 72 ib blocks into psum, copy to
    SBUF sY on ACT. Returns the sY tile."""
    f32 = mybir.dt.float32
    pss = psS.tile([96, ON], f32, tag="psS", name="pss")
    for ib in range(IB):
        nc.tensor.matmul(
            pss[0:80, :], cbd[:, :, ib], U[:, ib, :],
            start=(ib == 0), stop=(ib == IB - 1),
        )
    sY = small.tile([96, ON], f32, tag="sY", name=f"sY{c}_{it}", bufs=1)
    nc.vector.tensor_copy(sY[0:80, :], pss[0:80, :])
    return sY


def _tree(nc, mybir, tree, small, psW, U, vcur, c, it):
    """Agreement logits bb[p, ib, o] = sum_n U[p, ib, (o,n)] * vcur[p, (o,n)].

    Two halves of 36 ib each; mul + 4 halving adds, scratch ping-pong
    between SA and SB (all DVE, program-order serial on the engine).
    Tiny keep-warm matmuls chained to the scratch keep the PE HAM
    un-throttled through the DVE-heavy stretch.
    """
    f32 = mybir.dt.float32
    bf16 = mybir.dt.bfloat16

    bb = small.tile([128, IB, O], f32, tag="bb", name=f"bb{c}_{it}")
    vv = (
        vcur.rearrange("p (o n) -> p o n", n=N)
        .unsqueeze(1)
        .broadcast_to([128, HH, O, N])
    )
    for h in range(2):
        sa = tree.tile([128, HH, ON], bf16, tag="SA", name="sa")
        sb = tree.tile([128, HH, O, 8], bf16, tag="SB", name="sb")
        sa4 = sa[:].rearrange("p i (o n) -> p i o n", n=N)
        uh = U[:, h * HH:(h + 1) * HH, :].rearrange(
            "p i (o n) -> p i o n", n=N
        )
        nc.vector.tensor_mul(sa4, uh, vv)
        nc.vector.tensor_add(sb[:], sa4[:, :, :, 0:8], sa4[:, :, :, 8:16])
        nc.vector.tensor_add(
            sa4[:, :, :, 0:4], sb[:, :, :, 0:4], sb[:, :, :, 4:8]
        )
        nc.vector.tensor_add(
            sb[:, :, :, 0:2], sa4[:, :, :, 0:2], sa4[:, :, :, 2:4]
        )
        nc.vector.tensor_add(
            bb[:, h * HH:(h + 1) * HH, :], sb[:, :, :, 0], sb[:, :, :, 1]
        )
    return bb


def _warm(nc, mybir, psW, src2d):
    """Tiny matmul reading just-produced DVE scratch: keeps the PE HAM
    activity window busy during DVE-heavy stretches. Output is discarded."""
    f32 = mybir.dt.float32
    pw = psW.tile([16, 16], f32, tag="warm", name="warm")
    nc.tensor.matmul(
        pw[:], src2d[:, 0:16], src2d[:, 0:16], start=True, stop=True
    )


def _chain(nc, tc, mybir, small, tree, psW, U, cbd, vrep0s, vsums, out_d,
           sY, c, it, n_routing):
    """Post-matmul per-chunk work for iteration it: diag extract, squash,
    vrep build, running-vsum tree (logits), softmax, scatter. For it==0
    (no s-matmul; sY=None) uses the shared vrep0. For the last it, just
    squash and write the output."""
    f32 = mybir.dt.float32
    bf16 = mybir.dt.bfloat16
    AX = mybir.AxisListType
    OP = mybir.AluOpType
    AF = mybir.ActivationFunctionType
    last = it == n_routing - 1

    if it == 0:
        vrep = vrep0s[c]
    else:
        # diag extract via shuffles: s3[b, o, n] = sY[o*8+b, o, n]
        s3 = small.tile([32, ON], f32, tag="s3", name="s3", bufs=1)
        s3v = s3[:].rearrange("p (o n) -> p o n", n=N)
        sYv = sY[:].rearrange("p (o n) -> p o n", n=N)
        for o in range(O):
            g = o // 4
            nc.vector.stream_shuffle(
                s3v[0:32, o, :],
                sYv[g * 32:g * 32 + 32, o, :],
                [o * 8 - 32 * g + (p % 8) for p in range(32)],
            )
        s3b = s3[0:BC, :].rearrange("p (o n) -> p o n", n=N)
        sc = _squash(nc, mybir, small, BC, s3b)
        if last:
            v3f = small.tile([BC, O, N], f32, tag="v3f", name="v3f")
            nc.vector.tensor_mul(
                v3f[:], s3b,
                sc.unsqueeze(2).broadcast_to([BC, O, N]),
            )
            nc.sync.dma_start(
                out=out_d[c * BC:(c + 1) * BC, :, :], in_=v3f[:]
            )
            return
        v3b = small.tile([32, ON], bf16, tag="v3b", name="v3b")
        nc.vector.memset(v3b[:], 0.0)
        nc.vector.tensor_mul(
            v3b[0:BC, :].rearrange("p (o n) -> p o n", n=N),
            s3b,
            sc.unsqueeze(2).broadcast_to([BC, O, N]),
        )
        vrep = small.tile([128, ON], bf16, tag="vrep", name="vrep")
        for q in range(4):
            nc.vector.stream_shuffle(
                vrep[q * 32:(q + 1) * 32, :],
                v3b[:],
                [2 * q + (j // 16) for j in range(32)],
            )

    # ---------------- running vsum + agreement tree ---------------------
    if it == 0:
        vcur = vrep
    elif it == 1:
        vs = small.tile([128, ON], bf16, tag=f"vs{c}", name=f"vs{c}", bufs=1)
        nc.vector.tensor_add(vs[:], vrep0s[c][:], vrep[:])
        vsums[c] = vs
        vcur = vs
    else:
        vs = vsums[c]
        nc.vector.tensor_add(vs[:], vs[:], vrep[:])
        vcur = vs
    bb = _tree(nc, mybir, tree, small, psW, U, vcur[:], c, it)

    # ---------------- softmax over o -> scatter into cbd ----------------
    c2 = small.tile([128, O, IB], bf16, tag="c2", name="c2", bufs=1)
    nc.scalar.activation(c2[:].transpose([0, 2, 1]), bb[:], AF.Exp)
    ssum = small.tile([128, IB], f32, tag="ssum", name="ssum")
    nc.vector.tensor_reduce(
        ssum[:], c2[:].transpose([0, 2, 1]), axis=AX.X, op=OP.add
    )
    rs = small.tile([128, IB], f32, tag="rs", name="rs")
    nc.vector.reciprocal(rs[:], ssum[:])
    c2n = small.tile([128, O, IB], bf16, tag="c2n", name="c2n")
    nc.gpsimd.tensor_mul(
        c2n[:], c2[:], rs[:].unsqueeze(1).broadcast_to([128, O, IB])
    )
    for b in range(BC):
        nc.sync.dma_start(
            out=cbd[b * 16:(b + 1) * 16, b:80:8, :],
            in_=c2n[b * 16:(b + 1) * 16, :, :],
        )


_CACHE = {}


def _get(n_routing: int, reps: int = 1):
    key = (n_routing, reps)
    if key not in _CACHE:
        _CACHE[key] = _build(n_routing, reps)
    return _CACHE[key]


def _bf16(a):
    import ml_dtypes

    return np.asarray(a, dtype=ml_dtypes.bfloat16)


def _prep_host(inputs: np.ndarray, W: np.ndarray):
    x = np.ascontiguousarray(np.asarray(inputs, dtype=np.float32))
    W = np.asarray(W, dtype=np.float32)
    # w2n[(il,v), ib, (o,n)] = W[ib*16+il, o, v, n]
    w2n = np.ascontiguousarray(
        W.reshape(IB, 16, O, V, N).transpose(1, 3, 0, 2, 4).reshape(128, IB, O * N)
    )
    return x, _bf16(w2n)


def _make_in_maps(inputs, W):
    x, w2n = _prep_host(inputs, W)
    in_maps = []
    for core in range(NCORES):
        xc = x[core * BLOC:(core + 1) * BLOC]              # [32, 1152, 8]
        # xbdh[c, il*8+v, ib, b*16+il] = xc[c*BC+b, ib*16+il, v]
        xr = xc.reshape(NCHUNK, BC, IB, 16, V)
        xbdh = np.zeros((NCHUNK, 128, IB, 128), dtype=np.float32)
        for il in range(16):
            xbdh[:, il * 8:(il + 1) * 8, :, il::16] = xr[:, :, :, il, :].transpose(
                0, 3, 2, 1
            )
        # xt2[(il,v), ib, b] = xc[b, ib*16+il, v]
        xt2 = np.ascontiguousarray(
            xc.reshape(BLOC, IB, 16, V).transpose(2, 3, 1, 0).reshape(128, IB, BLOC)
        )
        in_maps.append(
            {"xbdh": _bf16(xbdh), "w2": w2n, "xt2": _bf16(xt2)}
        )
    return in_maps


def kernel(inputs, W, n_routing):
    from concourse.bass_utils import run_bass_kernel_spmd

    n_routing = int(n_routing)
    nc = _get(n_routing)
    in_maps = _make_in_maps(inputs, W)
    res = run_bass_kernel_spmd(nc, in_maps, core_ids=list(range(NCORES)))
    outs = [res.results[i]["out"] for i in range(NCORES)]
    return np.concatenate(outs, axis=0).astype(np.float32)


# revision 24
# speedup vs baseline: 1.1648x; 1.1648x over previous
"""CapsuleLayer (dynamic routing) Trainium2 Bass kernel — v3.

Math (per example b):
  u_hat[b,i,o,n] = sum_v x[b,i,v] * W[i,o,v,n]        I=1152, O=10, V=8, N=16
  b_logits = 0; repeat n_routing times:
    c = softmax_o(b_logits); s = sum_i c*u_hat; out = squash(s)
    if not last: b_logits += sum_n u_hat*out

Distribution: batch B=256 sharded over 8 cores (32 each). W replicated.

v3 structure (vs v2):
  - (o,n)-inner free layout: U[p=(b,il), ib, (o,n)] so phase-1 psum->SBUF
    copies are contiguous ACT copies and s-matmul rhs slices are contiguous.
  - logit linearity: b_t = u_hat . (v_0+..+v_{t-1}), so each iteration's
    tree uses the running vsum and writes the logits directly (no bb
    accumulate/copy passes).
  - coarse 2-half agreement trees (mul + 4 halving adds per half) on DVE
    with in-place scratch reuse; fewer per-op overheads.
  - phase-1 build interleaved with it0 routing so DVE trees overlap the
    PE build stream; xe slices prefetched (bufs=3) on alternating queues.
  - squash rsqrt: DVE bit-hack + 1 Newton step.

Per-core layout (chunk = 8 examples, 4 chunks), i = ib*16 + il:
  K partitions k = il*8+v   (contraction rows of the u_hat matmul)
  M partitions p = b*16+il  (rows of u_hat / routing state)
  U[c]  [128, 72, 160] bf16   U[(b,il), ib, (o,n)]
  XBD   [128, 9, 128] bf16 x stationary eighth: XBD[(il,v), ibl, (b,il')]
  xt2   [128, 72, 32] bf16 dense x stationary: xt2[(il,v), ib, b]
  w2n   [128, 72, 160] bf16  w2n[(il,v), ib, (o,n)] = W[ib*16+il, o, v, n]
  CBD[c] [128, 80, 72] bf16 block-diag c stationary: CBD[(b,il),(o,b'),ib]
  u_hat matmul (per ib): psum[(b,il'),(o,n)] = XBD[:,ibl,:].T @ w2n[:,ib,:]
  s matmul (per it,c): psum[(o,b'),(o',n)] += CBD[:,:,ib].T @ U[:,ib,:]
    -> diagonal o==o' holds s[b', o, n]  (extracted via DVE shuffles)
"""

import os
import sys

import numpy as np

_TRN_REPO = "/opt/trn_rl_repo"
if _TRN_REPO not in sys.path:
    sys.path.insert(0, _TRN_REPO)

EPS = 1e-10
B, I, V, O, N = 256, 1152, 8, 10, 16
NCORES = 8
BLOC = B // NCORES          # 32 examples per core
BC = 8                      # examples per chunk
NCHUNK = BLOC // BC         # 4
IB = I // 16                # 72 i-blocks
ON = O * N                  # 160
EI = 8                      # xbd load slices per chunk
IBE = IB // EI              # 9 i-blocks per slice
HH = IB // 2                # 36 i-blocks per tree half
RSQRT_MAGIC = 0x5F3759DF
NEWTON = 1


def _squash(nc, mybir, small, sv, nsq_src):
    """Emit the squash scale chain, fully on DVE (tiny ops; avoids
    cross-engine ping-pong latency in the per-chunk chains).

    nsq_src: AP [sv, O, N] view (strided ok) of s to square-reduce.
    Returns sc AP [sv, O] (the per-(b,o) scale nsq*rsqrt(...)).
    """
    f32 = mybir.dt.float32
    i32 = mybir.dt.int32
    AX = mybir.AxisListType
    OP = mybir.AluOpType

    sq = small.tile([sv, O, N], f32, tag="sq", name="sq")
    nc.vector.tensor_mul(sq[:], nsq_src, nsq_src)
    nsq = small.tile([sv, O], f32, tag="nsq", name="nsq")
    nc.vector.tensor_reduce(nsq[:], sq[:], axis=AX.X, op=OP.add)
    np1 = small.tile([sv, O], f32, tag="np1", name="np1")
    nc.gpsimd.tensor_scalar_add(np1[:], nsq[:], 1.0)
    d1 = small.tile([sv, O], f32, tag="d1", name="d1")
    nc.gpsimd.tensor_mul(d1[:], np1[:], np1[:])
    nse = small.tile([sv, O], f32, tag="nse", name="nse")
    nc.gpsimd.tensor_scalar_add(nse[:], nsq[:], EPS)
    dd = small.tile([sv, O], f32, tag="dd", name="dd")
    nc.gpsimd.tensor_mul(dd[:], d1[:], nse[:])
    yy = small.tile([sv, O], f32, tag="yy", name="yy")
    nc.vector.tensor_scalar(
        yy[:].bitcast(i32), dd[:].bitcast(i32), 1, None,
        op0=OP.logical_shift_right,
    )
    nc.vector.tensor_scalar(
        yy[:].bitcast(i32), yy[:].bitcast(i32), -1, RSQRT_MAGIC,
        op0=OP.mult, op1=OP.add,
    )
    for _ in range(NEWTON):
        y2 = small.tile([sv, O], f32, tag="y2", name="y2")
        nc.gpsimd.tensor_mul(y2[:], yy[:], yy[:])
        t2 = small.tile([sv, O], f32, tag="t2", name="t2")
        nc.gpsimd.tensor_mul(t2[:], y2[:], dd[:])
        u2 = small.tile([sv, O], f32, tag="u2", name="u2")
        nc.gpsimd.tensor_scalar(
            u2[:], t2[:], -0.5, 1.5, op0=OP.mult, op1=OP.add
        )
        yn = small.tile([sv, O], f32, tag="yn", name="yn")
        nc.gpsimd.tensor_mul(yn[:], yy[:], u2[:])
        yy = yn
    sc = small.tile([sv, O], f32, tag="sc", name="sc")
    nc.vector.tensor_mul(sc[:], nsq[:], yy[:])
    return sc


def _build(n_routing: int, reps: int = 1):
    import concourse.bacc as bacc
    import concourse.tile as tile
    from concourse import mybir

    nc = bacc.Bacc("TRN2", target_bir_lowering=False, debug=False)
    f32 = mybir.dt.float32
    bf16 = mybir.dt.bfloat16

    xbdh = nc.dram_tensor(
        "xbdh", [NCHUNK, 128, IB, 128], bf16, kind="ExternalInput"
    )
    w2 = nc.dram_tensor("w2", [128, IB, ON], bf16, kind="ExternalInput")
    xt2 = nc.dram_tensor("xt2", [128, IB, BLOC], bf16, kind="ExternalInput")
    out_d = nc.dram_tensor("out", [BLOC, O, N], f32, kind="ExternalOutput")

    with tile.TileContext(nc) as tc:
        with (
            tc.tile_pool(name="state", bufs=1) as state,
            tc.tile_pool(name="small", bufs=2) as small,
            tc.tile_pool(name="tree", bufs=1) as tree,
            tc.tile_pool(name="xep", bufs=3) as xep,
            tc.tile_pool(name="psA", bufs=3, space="PSUM") as psA,
            tc.tile_pool(name="psR", bufs=1, space="PSUM") as psR,
            tc.tile_pool(name="psS", bufs=2, space="PSUM") as psS,
            tc.tile_pool(name="psW", bufs=1, space="PSUM") as psW,
        ):
            xt2s = state.tile([128, IB, BLOC], bf16)
            nc.gpsimd.dma_start(out=xt2s[:], in_=xt2[:])
            # w2 in quarters so it0 matmuls start as soon as data lands
            IBQ = IB // 4
            w2qs = []
            for k in range(4):
                w2q = state.tile([128, IBQ, ON], bf16, tag=f"w2q{k}")
                nc.sync.dma_start(
                    out=w2q[:], in_=w2[:, k * IBQ:(k + 1) * IBQ, :]
                )
                w2qs.append(w2q)

            def w2sl(ib):
                return w2qs[ib // IBQ][:, ib % IBQ, :]

            Us = [
                state.tile([128, IB, ON], bf16, tag=f"U{c}", name=f"U{c}")
                for c in range(NCHUNK)
            ]
            cbds = [
                state.tile([128, 80, IB], bf16, tag=f"cbd{c}", name=f"cbd{c}")
                for c in range(NCHUNK)
            ] if n_routing > 1 else []
            for cb in cbds:
                nc.gpsimd.memset(cb[:], 0.0)

            for rep in range(reps):
                _body(
                    nc, tc, mybir, tile, state, small, tree, xep,
                    psA, psR, psS, psW, xbdh, w2sl, xt2s, Us, cbds,
                    out_d, n_routing,
                )

    nc.compile()
    return nc


def _phase1(nc, mybir, xep, psA, xbdh, w2sl, Us, c):
    f32 = mybir.dt.float32
    bf16 = mybir.dt.bfloat16
    for e in range(EI):
        xe = xep.tile([128, IBE, 128], bf16, tag="xbd", name="xe")
        q = nc.gpsimd if e % 2 == 0 else nc.sync
        q.dma_start(out=xe[:], in_=xbdh[c, :, e * IBE:(e + 1) * IBE, :])
        for g in range(IBE // 3):
            ps = psA.tile([128, 3, ON], f32, tag="psA", name="psa")
            for j in range(3):
                ibl = g * 3 + j
                ib = e * IBE + ibl
                nc.tensor.matmul(
                    ps[:, j, :], xe[:, ibl, :], w2sl(ib),
                    start=True, stop=True,
                )
            ib0 = e * IBE + g * 3
            # contiguous flattened [128, 480] f32 -> bf16 copy on ACT
            dst = Us[c][:, ib0:ib0 + 3, :].rearrange("p a b -> p (a b)")
            src = ps[:].rearrange("p a b -> p (a b)")
            nc.scalar.copy(dst, src)


def _body(nc, tc, mybir, tile, state, small, tree, xep, psA, psR, psS, psW,
          xbdh, w2sl, xt2s, Us, cbds, out_d, n_routing):
    f32 = mybir.dt.float32
    bf16 = mybir.dt.bfloat16
    AX = mybir.AxisListType
    OP = mybir.AluOpType
    AF = mybir.ActivationFunctionType

    # ---------------- iteration-0 s shortcut: r = sum_i u_hat ------------
    # psR[b, (o,n)] = sum_ib xt2s[:, ib, :].T @ w2n[:, ib, :]
    pr = psR.tile([BLOC, ON], f32, tag="psR", name="pr")
    for ib in range(IB):
        nc.tensor.matmul(
            pr[:], xt2s[:, ib, :], w2sl(ib),
            start=(ib == 0), stop=(ib == IB - 1),
        )
    s0 = small.tile([BLOC, ON], f32, tag="s0", name="s0")
    nc.scalar.mul(s0[:], pr[:], 1.0 / O)
    # squash over n for all 32 examples; s0 layout (o, n)
    s0v = s0[:].rearrange("p (o n) -> p o n", n=N)
    sc0 = _squash(nc, mybir, small, BLOC, s0v)
    v3b0 = small.tile([BLOC, ON], bf16, tag="v3b0", name="v3b0")
    nc.vector.tensor_mul(
        v3b0[:].rearrange("p (o n) -> p o n", n=N),
        s0v,
        sc0.unsqueeze(2).broadcast_to([BLOC, O, N]),
    )
    vrep0s = []
    for c in range(NCHUNK):
        vrep = small.tile(
            [128, ON], bf16, tag=f"vrep0_{c}", name=f"vr0_{c}", bufs=1
        )
        for q in range(4):
            nc.vector.stream_shuffle(
                vrep[q * 32:(q + 1) * 32, :],
                v3b0[:],
                [c * BC + 2 * q + (j // 16) for j in range(32)],
            )
        vrep0s.append(vrep)

    # ---------------- phase 1 (u_hat) interleaved with it0 routing -------
    vsums = [None] * NCHUNK
    _phase1(nc, mybir, xep, psA, xbdh, w2sl, Us, 0)
    _phase1(nc, mybir, xep, psA, xbdh, w2sl, Us, 1)
    _chain(nc, tc, mybir, small, tree, psW, Us[0],
           cbds[0] if cbds else None, vrep0s, vsums, out_d, None,
           0, 0, n_routing)
    _phase1(nc, mybir, xep, psA, xbdh, w2sl, Us, 2)
    _chain(nc, tc, mybir, small, tree, psW, Us[1],
           cbds[1] if cbds else None, vrep0s, vsums, out_d, None,
           1, 0, n_routing)
    _phase1(nc, mybir, xep, psA, xbdh, w2sl, Us, 3)
    _chain(nc, tc, mybir, small, tree, psW, Us[2],
           cbds[2] if cbds else None, vrep0s, vsums, out_d, None,
           2, 0, n_routing)
    _chain(nc, tc, mybir, small, tree, psW, Us[3],
           cbds[3] if cbds else None, vrep0s, vsums, out_d, None,
           3, 0, n_routing)

    for it in range(1, n_routing):
        for c in range(NCHUNK):
            sY = _smm(nc, mybir, small, psS, Us[c], cbds[c], c, it)
            _chain(nc, tc, mybir, small, tree, psW, Us[c], cbds[c],
                   vrep0s, vsums, out_d, sY, c, it, n_routing)


def _smm(nc, mybir, small, psS, U, cbd, c, it):
    """s matmul for (it, c): accumulate 72 ib blocks into psum, copy to
    SBUF sY on ACT. Returns the sY tile."""
    f32 = mybir.dt.float32
    pss = psS.tile([96, ON], f32, tag="psS", name="pss")
    for ib in range(IB):
        nc.tensor.matmul(
            pss[0:80, :], cbd[:, :, ib], U[:, ib, :],
            start=(ib == 0), stop=(ib == IB - 1),
        )
    sY = small.tile([96, ON], f32, tag="sY", name=f"sY{c}_{it}", bufs=1)
    nc.scalar.copy(sY[0:80, :], pss[0:80, :])
    return sY


def _tree(nc, mybir, tree, small, psW, U, vcur, c, it):
    """Agreement logits bb[p, ib, o] = sum_n U[p, ib, (o,n)] * vcur[p, (o,n)].

    Two halves of 36 ib each; mul + 4 halving adds, scratch ping-pong
    between SA and SB (all DVE, program-order serial on the engine).
    Tiny keep-warm matmuls chained to the scratch keep the PE HAM
    un-throttled through the DVE-heavy stretch.
    """
    f32 = mybir.dt.float32
    bf16 = mybir.dt.bfloat16

    bb = small.tile([128, IB, O], f32, tag="bb", name=f"bb{c}_{it}")
    vv = (
        vcur.rearrange("p (o n) -> p o n", n=N)
        .unsqueeze(1)
        .broadcast_to([128, HH, O, N])
    )
    for h in range(2):
        sa = tree.tile([128, HH, ON], bf16, tag="SA", name="sa")
        sb = tree.tile([128, HH, O, 8], bf16, tag="SB", name="sb")
        sa4 = sa[:].rearrange("p i (o n) -> p i o n", n=N)
        uh = U[:, h * HH:(h + 1) * HH, :].rearrange(
            "p i (o n) -> p i o n", n=N
        )
        nc.vector.tensor_mul(sa4, uh, vv)
        nc.vector.tensor_add(sb[:], sa4[:, :, :, 0:8], sa4[:, :, :, 8:16])
        nc.vector.tensor_add(
            sa4[:, :, :, 0:4], sb[:, :, :, 0:4], sb[:, :, :, 4:8]
        )
        nc.vector.tensor_add(
            sb[:, :, :, 0:2], sa4[:, :, :, 0:2], sa4[:, :, :, 2:4]
        )
        nc.vector.tensor_add(
            bb[:, h * HH:(h + 1) * HH, :], sb[:, :, :, 0], sb[:, :, :, 1]
        )
    return bb


def _warm(nc, mybir, psW, src2d):
    """Tiny matmul reading just-produced DVE scratch: keeps the PE HAM
    activity window busy during DVE-heavy stretches. Output is discarded."""
    f32 = mybir.dt.float32
    pw = psW.tile([16, 16], f32, tag="warm", name="warm")
    nc.tensor.matmul(
        pw[:], src2d[:, 0:16], src2d[:, 0:16], start=True, stop=True
    )


def _chain(nc, tc, mybir, small, tree, psW, U, cbd, vrep0s, vsums, out_d,
           sY, c, it, n_routing):
    """Post-matmul per-chunk work for iteration it: diag extract, squash,
    vrep build, running-vsum tree (logits), softmax, scatter. For it==0
    (no s-matmul; sY=None) uses the shared vrep0. For the last it, just
    squash and write the output."""
    f32 = mybir.dt.float32
    bf16 = mybir.dt.bfloat16
    AX = mybir.AxisListType
    OP = mybir.AluOpType
    AF = mybir.ActivationFunctionType
    last = it == n_routing - 1

    if it == 0:
        vrep = vrep0s[c]
    else:
        # diag extract via shuffles: s3[b, o, n] = sY[o*8+b, o, n]
        s3 = small.tile([32, ON], f32, tag="s3", name="s3", bufs=1)
        s3v = s3[:].rearrange("p (o n) -> p o n", n=N)
        sYv = sY[:].rearrange("p (o n) -> p o n", n=N)
        for o in range(O):
            g = o // 4
            nc.vector.stream_shuffle(
                s3v[0:32, o, :],
                sYv[g * 32:g * 32 + 32, o, :],
                [o * 8 - 32 * g + (p % 8) for p in range(32)],
            )
        s3b = s3[0:BC, :].rearrange("p (o n) -> p o n", n=N)
        sc = _squash(nc, mybir, small, BC, s3b)
        if last:
            v3f = small.tile([BC, O, N], f32, tag="v3f", name="v3f")
            nc.vector.tensor_mul(
                v3f[:], s3b,
                sc.unsqueeze(2).broadcast_to([BC, O, N]),
            )
            nc.sync.dma_start(
                out=out_d[c * BC:(c + 1) * BC, :, :], in_=v3f[:]
            )
            return
        v3b = small.tile([32, ON], bf16, tag="v3b", name="v3b")
        nc.vector.memset(v3b[:], 0.0)
        nc.vector.tensor_mul(
            v3b[0:BC, :].rearrange("p (o n) -> p o n", n=N),
            s3b,
            sc.unsqueeze(2).broadcast_to([BC, O, N]),
        )
        vrep = small.tile([128, ON], bf16, tag="vrep", name="vrep")
        for q in range(4):
            nc.vector.stream_shuffle(
                vrep[q * 32:(q + 1) * 32, :],
                v3b[:],
                [2 * q + (j // 16) for j in range(32)],
            )

    # ---------------- running vsum + agreement tree ---------------------
    if it == 0:
        vcur = vrep
    elif it == 1:
        vs = small.tile([128, ON], bf16, tag=f"vs{c}", name=f"vs{c}", bufs=1)
        nc.vector.tensor_add(vs[:], vrep0s[c][:], vrep[:])
        vsums[c] = vs
        vcur = vs
    else:
        vs = vsums[c]
        nc.vector.tensor_add(vs[:], vs[:], vrep[:])
        vcur = vs
    bb = _tree(nc, mybir, tree, small, psW, U, vcur[:], c, it)

    # ---------------- softmax over o -> scatter into cbd ----------------
    c2 = small.tile([128, O, IB], bf16, tag="c2", name="c2", bufs=1)
    nc.scalar.activation(c2[:].transpose([0, 2, 1]), bb[:], AF.Exp)
    ssum = small.tile([128, IB], f32, tag="ssum", name="ssum")
    nc.vector.tensor_reduce(
        ssum[:], c2[:].transpose([0, 2, 1]), axis=AX.X, op=OP.add
    )
    rs = small.tile([128, IB], f32, tag="rs", name="rs")
    nc.vector.reciprocal(rs[:], ssum[:])
    c2n = small.tile([128, O, IB], bf16, tag="c2n", name="c2n")
    nc.gpsimd.tensor_mul(
        c2n[:], c2[:], rs[:].unsqueeze(1).broadcast_to([128, O, IB])
    )
    for b in range(BC):
        nc.sync.dma_start(
            out=cbd[b * 16:(b + 1) * 16, b:80:8, :],
            in_=c2n[b * 16:(b + 1) * 16, :, :],
        )


_CACHE = {}


def _get(n_routing: int, reps: int = 1):
    key = (n_routing, reps)
    if key not in _CACHE:
        _CACHE[key] = _build(n_routing, reps)
    return _CACHE[key]


def _bf16(a):
    import ml_dtypes

    return np.asarray(a, dtype=ml_dtypes.bfloat16)


def _prep_host(inputs: np.ndarray, W: np.ndarray):
    x = np.ascontiguousarray(np.asarray(inputs, dtype=np.float32))
    W = np.asarray(W, dtype=np.float32)
    # w2n[(il,v), ib, (o,n)] = W[ib*16+il, o, v, n]
    w2n = np.ascontiguousarray(
        W.reshape(IB, 16, O, V, N).transpose(1, 3, 0, 2, 4).reshape(128, IB, O * N)
    )
    return x, _bf16(w2n)


def _make_in_maps(inputs, W):
    x, w2n = _prep_host(inputs, W)
    in_maps = []
    for core in range(NCORES):
        xc = x[core * BLOC:(core + 1) * BLOC]              # [32, 1152, 8]
        # xbdh[c, il*8+v, ib, b*16+il] = xc[c*BC+b, ib*16+il, v]
        xr = xc.reshape(NCHUNK, BC, IB, 16, V)
        xbdh = np.zeros((NCHUNK, 128, IB, 128), dtype=np.float32)
        for il in range(16):
            xbdh[:, il * 8:(il + 1) * 8, :, il::16] = xr[:, :, :, il, :].transpose(
                0, 3, 2, 1
            )
        # xt2[(il,v), ib, b] = xc[b, ib*16+il, v]
        xt2 = np.ascontiguousarray(
            xc.reshape(BLOC, IB, 16, V).transpose(2, 3, 1, 0).reshape(128, IB, BLOC)
        )
        in_maps.append(
            {"xbdh": _bf16(xbdh), "w2": w2n, "xt2": _bf16(xt2)}
        )
    return in_maps


def kernel(inputs, W, n_routing):
    from concourse.bass_utils import run_bass_kernel_spmd

    n_routing = int(n_routing)
    nc = _get(n_routing)
    in_maps = _make_in_maps(inputs, W)
    res = run_bass_kernel_spmd(nc, in_maps, core_ids=list(range(NCORES)))
    outs = [res.results[i]["out"] for i in range(NCORES)]
    return np.concatenate(outs, axis=0).astype(np.float32)


# revision 25
# speedup vs baseline: 1.2051x; 1.0346x over previous
"""CapsuleLayer (dynamic routing) Trainium2 Bass kernel — v3.

Math (per example b):
  u_hat[b,i,o,n] = sum_v x[b,i,v] * W[i,o,v,n]        I=1152, O=10, V=8, N=16
  b_logits = 0; repeat n_routing times:
    c = softmax_o(b_logits); s = sum_i c*u_hat; out = squash(s)
    if not last: b_logits += sum_n u_hat*out

Distribution: batch B=256 sharded over 8 cores (32 each). W replicated.

v3 structure (vs v2):
  - (o,n)-inner free layout: U[p=(b,il), ib, (o,n)] so phase-1 psum->SBUF
    copies are contiguous ACT copies and s-matmul rhs slices are contiguous.
  - logit linearity: b_t = u_hat . (v_0+..+v_{t-1}), so each iteration's
    tree uses the running vsum and writes the logits directly (no bb
    accumulate/copy passes).
  - coarse 2-half agreement trees (mul + 4 halving adds per half) on DVE
    with in-place scratch reuse; fewer per-op overheads.
  - phase-1 build interleaved with it0 routing so DVE trees overlap the
    PE build stream; xe slices prefetched (bufs=3) on alternating queues.
  - squash rsqrt: DVE bit-hack + 1 Newton step.

Per-core layout (chunk = 8 examples, 4 chunks), i = ib*16 + il:
  K partitions k = il*8+v   (contraction rows of the u_hat matmul)
  M partitions p = b*16+il  (rows of u_hat / routing state)
  U[c]  [128, 72, 160] bf16   U[(b,il), ib, (o,n)]
  XBD   [128, 9, 128] bf16 x stationary eighth: XBD[(il,v), ibl, (b,il')]
  xt2   [128, 72, 32] bf16 dense x stationary: xt2[(il,v), ib, b]
  w2n   [128, 72, 160] bf16  w2n[(il,v), ib, (o,n)] = W[ib*16+il, o, v, n]
  CBD[c] [128, 80, 72] bf16 block-diag c stationary: CBD[(b,il),(o,b'),ib]
  u_hat matmul (per ib): psum[(b,il'),(o,n)] = XBD[:,ibl,:].T @ w2n[:,ib,:]
  s matmul (per it,c): psum[(o,b'),(o',n)] += CBD[:,:,ib].T @ U[:,ib,:]
    -> diagonal o==o' holds s[b', o, n]  (extracted via DVE shuffles)
"""

import os
import sys

import numpy as np

_TRN_REPO = "/opt/trn_rl_repo"
if _TRN_REPO not in sys.path:
    sys.path.insert(0, _TRN_REPO)

EPS = 1e-10
B, I, V, O, N = 256, 1152, 8, 10, 16
NCORES = 8
BLOC = B // NCORES          # 32 examples per core
BC = 8                      # examples per chunk
NCHUNK = BLOC // BC         # 4
IB = I // 16                # 72 i-blocks
ON = O * N                  # 160
EI = 8                      # xbd load slices per chunk
IBE = IB // EI              # 9 i-blocks per slice
HH = IB // 2                # 36 i-blocks per tree half
RSQRT_MAGIC = 0x5F3759DF
NEWTON = 1


def _squash(nc, mybir, small, sv, nsq_src):
    """Emit the squash scale chain, fully on DVE (tiny ops; avoids
    cross-engine ping-pong latency in the per-chunk chains).

    nsq_src: AP [sv, O, N] view (strided ok) of s to square-reduce.
    Returns sc AP [sv, O] (the per-(b,o) scale nsq*rsqrt(...)).
    """
    f32 = mybir.dt.float32
    i32 = mybir.dt.int32
    AX = mybir.AxisListType
    OP = mybir.AluOpType

    sq = small.tile([sv, O, N], f32, tag="sq", name="sq")
    nc.vector.tensor_mul(sq[:], nsq_src, nsq_src)
    nsq = small.tile([sv, O], f32, tag="nsq", name="nsq")
    nc.vector.tensor_reduce(nsq[:], sq[:], axis=AX.X, op=OP.add)
    np1 = small.tile([sv, O], f32, tag="np1", name="np1")
    nc.gpsimd.tensor_scalar_add(np1[:], nsq[:], 1.0)
    d1 = small.tile([sv, O], f32, tag="d1", name="d1")
    nc.gpsimd.tensor_mul(d1[:], np1[:], np1[:])
    nse = small.tile([sv, O], f32, tag="nse", name="nse")
    nc.gpsimd.tensor_scalar_add(nse[:], nsq[:], EPS)
    dd = small.tile([sv, O], f32, tag="dd", name="dd")
    nc.gpsimd.tensor_mul(dd[:], d1[:], nse[:])
    yy = small.tile([sv, O], f32, tag="yy", name="yy")
    nc.vector.tensor_scalar(
        yy[:].bitcast(i32), dd[:].bitcast(i32), 1, None,
        op0=OP.logical_shift_right,
    )
    nc.vector.tensor_scalar(
        yy[:].bitcast(i32), yy[:].bitcast(i32), -1, RSQRT_MAGIC,
        op0=OP.mult, op1=OP.add,
    )
    for _ in range(NEWTON):
        y2 = small.tile([sv, O], f32, tag="y2", name="y2")
        nc.gpsimd.tensor_mul(y2[:], yy[:], yy[:])
        t2 = small.tile([sv, O], f32, tag="t2", name="t2")
        nc.gpsimd.tensor_mul(t2[:], y2[:], dd[:])
        u2 = small.tile([sv, O], f32, tag="u2", name="u2")
        nc.gpsimd.tensor_scalar(
            u2[:], t2[:], -0.5, 1.5, op0=OP.mult, op1=OP.add
        )
        yn = small.tile([sv, O], f32, tag="yn", name="yn")
        nc.gpsimd.tensor_mul(yn[:], yy[:], u2[:])
        yy = yn
    sc = small.tile([sv, O], f32, tag="sc", name="sc")
    nc.vector.tensor_mul(sc[:], nsq[:], yy[:])
    return sc


def _build(n_routing: int, reps: int = 1):
    import concourse.bacc as bacc
    import concourse.tile as tile
    from concourse import mybir

    nc = bacc.Bacc("TRN2", target_bir_lowering=False, debug=False)
    f32 = mybir.dt.float32
    bf16 = mybir.dt.bfloat16

    xbdh = nc.dram_tensor(
        "xbdh", [NCHUNK, 128, IB, 128], bf16, kind="ExternalInput"
    )
    w2 = nc.dram_tensor("w2", [128, IB, ON], bf16, kind="ExternalInput")
    xt2 = nc.dram_tensor("xt2", [128, IB, BLOC], bf16, kind="ExternalInput")
    out_d = nc.dram_tensor("out", [BLOC, O, N], f32, kind="ExternalOutput")

    with tile.TileContext(nc) as tc:
        with (
            tc.tile_pool(name="state", bufs=1) as state,
            tc.tile_pool(name="small", bufs=2) as small,
            tc.tile_pool(name="tree", bufs=1) as tree,
            tc.tile_pool(name="xep", bufs=3) as xep,
            tc.tile_pool(name="psA", bufs=3, space="PSUM") as psA,
            tc.tile_pool(name="psR", bufs=1, space="PSUM") as psR,
            tc.tile_pool(name="psS", bufs=2, space="PSUM") as psS,
        ):
            w2s = state.tile([128, IB, ON], bf16)
            nc.sync.dma_start(out=w2s[:], in_=w2[:])
            xt2s = state.tile([128, IB, BLOC], bf16)
            nc.sync.dma_start(out=xt2s[:], in_=xt2[:])

            def w2sl(ib):
                return w2s[:, ib, :]

            Us = [
                state.tile([128, IB, ON], bf16, tag=f"U{c}", name=f"U{c}")
                for c in range(NCHUNK)
            ]
            cbds = [
                state.tile([128, 80, IB], bf16, tag=f"cbd{c}", name=f"cbd{c}")
                for c in range(NCHUNK)
            ] if n_routing > 1 else []
            for cb in cbds:
                nc.gpsimd.memset(cb[:], 0.0)

            for rep in range(reps):
                _body(
                    nc, tc, mybir, tile, state, small, tree, xep,
                    psA, psR, psS, None, xbdh, w2sl, xt2s, Us, cbds,
                    out_d, n_routing,
                )

    nc.compile()
    return nc


def _phase1(nc, mybir, xep, psA, xbdh, w2sl, Us, c):
    f32 = mybir.dt.float32
    bf16 = mybir.dt.bfloat16
    for e in range(EI):
        xe = xep.tile([128, IBE, 128], bf16, tag="xbd", name="xe")
        q = nc.gpsimd if e % 2 == 0 else nc.sync
        q.dma_start(out=xe[:], in_=xbdh[c, :, e * IBE:(e + 1) * IBE, :])
        for g in range(IBE // 3):
            ps = psA.tile([128, 3, ON], f32, tag="psA", name="psa")
            for j in range(3):
                ibl = g * 3 + j
                ib = e * IBE + ibl
                nc.tensor.matmul(
                    ps[:, j, :], xe[:, ibl, :], w2sl(ib),
                    start=True, stop=True,
                )
            ib0 = e * IBE + g * 3
            # contiguous [128, 480] f32 -> bf16 copy on ACT
            nc.scalar.copy(Us[c][:, ib0:ib0 + 3, :], ps[:])


def _body(nc, tc, mybir, tile, state, small, tree, xep, psA, psR, psS, psW,
          xbdh, w2sl, xt2s, Us, cbds, out_d, n_routing):
    f32 = mybir.dt.float32
    bf16 = mybir.dt.bfloat16
    AX = mybir.AxisListType
    OP = mybir.AluOpType
    AF = mybir.ActivationFunctionType

    # ---------------- iteration-0 s shortcut: r = sum_i u_hat ------------
    # psR[b, (o,n)] = sum_ib xt2s[:, ib, :].T @ w2n[:, ib, :]
    pr = psR.tile([BLOC, ON], f32, tag="psR", name="pr")
    for ib in range(IB):
        nc.tensor.matmul(
            pr[:], xt2s[:, ib, :], w2sl(ib),
            start=(ib == 0), stop=(ib == IB - 1),
        )
    s0 = small.tile([BLOC, ON], f32, tag="s0", name="s0")
    nc.scalar.mul(s0[:], pr[:], 1.0 / O)
    # squash over n for all 32 examples; s0 layout (o, n)
    s0v = s0[:].rearrange("p (o n) -> p o n", n=N)
    sc0 = _squash(nc, mybir, small, BLOC, s0v)
    v3b0 = small.tile([BLOC, ON], bf16, tag="v3b0", name="v3b0")
    nc.vector.tensor_mul(
        v3b0[:].rearrange("p (o n) -> p o n", n=N),
        s0v,
        sc0.unsqueeze(2).broadcast_to([BLOC, O, N]),
    )
    vrep0s = []
    for c in range(NCHUNK):
        vrep = small.tile(
            [128, ON], bf16, tag=f"vrep0_{c}", name=f"vr0_{c}", bufs=1
        )
        for q in range(4):
            nc.vector.stream_shuffle(
                vrep[q * 32:(q + 1) * 32, :],
                v3b0[:],
                [c * BC + 2 * q + (j // 16) for j in range(32)],
            )
        vrep0s.append(vrep)

    # ---------------- phase 1 (u_hat) interleaved with it0 routing -------
    vsums = [None] * NCHUNK
    _phase1(nc, mybir, xep, psA, xbdh, w2sl, Us, 0)
    _phase1(nc, mybir, xep, psA, xbdh, w2sl, Us, 1)
    _chain(nc, tc, mybir, small, tree, psW, Us[0],
           cbds[0] if cbds else None, vrep0s, vsums, out_d, None,
           0, 0, n_routing)
    _phase1(nc, mybir, xep, psA, xbdh, w2sl, Us, 2)
    _chain(nc, tc, mybir, small, tree, psW, Us[1],
           cbds[1] if cbds else None, vrep0s, vsums, out_d, None,
           1, 0, n_routing)
    _phase1(nc, mybir, xep, psA, xbdh, w2sl, Us, 3)
    _chain(nc, tc, mybir, small, tree, psW, Us[2],
           cbds[2] if cbds else None, vrep0s, vsums, out_d, None,
           2, 0, n_routing)
    _chain(nc, tc, mybir, small, tree, psW, Us[3],
           cbds[3] if cbds else None, vrep0s, vsums, out_d, None,
           3, 0, n_routing)

    for it in range(1, n_routing):
        for c in range(NCHUNK):
            sY = _smm(nc, mybir, small, psS, Us[c], cbds[c], c, it)
            _chain(nc, tc, mybir, small, tree, psW, Us[c], cbds[c],
                   vrep0s, vsums, out_d, sY, c, it, n_routing)


def _smm(nc, mybir, small, psS, U, cbd, c, it):
    """s matmul for (it, c): accumulate 72 ib blocks into psum, copy to
    SBUF sY on ACT. Returns the sY tile."""
    f32 = mybir.dt.float32
    pss = psS.tile([96, ON], f32, tag="psS", name="pss")
    for ib in range(IB):
        nc.tensor.matmul(
            pss[0:80, :], cbd[:, :, ib], U[:, ib, :],
            start=(ib == 0), stop=(ib == IB - 1),
        )
    sY = small.tile([96, ON], f32, tag="sY", name=f"sY{c}_{it}", bufs=1)
    nc.scalar.copy(sY[0:80, :], pss[0:80, :])
    return sY


def _tree(nc, mybir, tree, small, psW, U, vcur, c, it):
    """Agreement logits bb[p, ib, o] = sum_n U[p, ib, (o,n)] * vcur[p, (o,n)].

    Two halves of 36 ib each; mul + 4 halving adds, scratch ping-pong
    between SA and SB (all DVE, program-order serial on the engine).
    Tiny keep-warm matmuls chained to the scratch keep the PE HAM
    un-throttled through the DVE-heavy stretch.
    """
    f32 = mybir.dt.float32
    bf16 = mybir.dt.bfloat16

    bb = small.tile([128, IB, O], f32, tag="bb", name=f"bb{c}_{it}")
    vv = (
        vcur.rearrange("p (o n) -> p o n", n=N)
        .unsqueeze(1)
        .broadcast_to([128, HH, O, N])
    )
    for h in range(2):
        sa = tree.tile([128, HH, ON], bf16, tag="SA", name="sa")
        sb = tree.tile([128, HH, O, 8], bf16, tag="SB", name="sb")
        sa4 = sa[:].rearrange("p i (o n) -> p i o n", n=N)
        uh = U[:, h * HH:(h + 1) * HH, :].rearrange(
            "p i (o n) -> p i o n", n=N
        )
        nc.vector.tensor_mul(sa4, uh, vv)
        nc.vector.tensor_add(sb[:], sa4[:, :, :, 0:8], sa4[:, :, :, 8:16])
        nc.vector.tensor_add(
            sa4[:, :, :, 0:4], sb[:, :, :, 0:4], sb[:, :, :, 4:8]
        )
        nc.vector.tensor_add(
            sb[:, :, :, 0:2], sa4[:, :, :, 0:2], sa4[:, :, :, 2:4]
        )
        nc.vector.tensor_add(
            bb[:, h * HH:(h + 1) * HH, :], sb[:, :, :, 0], sb[:, :, :, 1]
        )
    return bb


def _warm(nc, mybir, psW, src2d):
    """Tiny matmul reading just-produced DVE scratch: keeps the PE HAM
    activity window busy during DVE-heavy stretches. Output is discarded."""
    f32 = mybir.dt.float32
    pw = psW.tile([16, 16], f32, tag="warm", name="warm")
    nc.tensor.matmul(
        pw[:], src2d[:, 0:16], src2d[:, 0:16], start=True, stop=True
    )


def _chain(nc, tc, mybir, small, tree, psW, U, cbd, vrep0s, vsums, out_d,
           sY, c, it, n_routing):
    """Post-matmul per-chunk work for iteration it: diag extract, squash,
    vrep build, running-vsum tree (logits), softmax, scatter. For it==0
    (no s-matmul; sY=None) uses the shared vrep0. For the last it, just
    squash and write the output."""
    f32 = mybir.dt.float32
    bf16 = mybir.dt.bfloat16
    AX = mybir.AxisListType
    OP = mybir.AluOpType
    AF = mybir.ActivationFunctionType
    last = it == n_routing - 1

    if it == 0:
        vrep = vrep0s[c]
    else:
        # diag extract via shuffles: s3[b, o, n] = sY[o*8+b, o, n]
        s3 = small.tile([32, ON], f32, tag="s3", name="s3", bufs=1)
        s3v = s3[:].rearrange("p (o n) -> p o n", n=N)
        sYv = sY[:].rearrange("p (o n) -> p o n", n=N)
        for o in range(O):
            g = o // 4
            nc.vector.stream_shuffle(
                s3v[0:32, o, :],
                sYv[g * 32:g * 32 + 32, o, :],
                [o * 8 - 32 * g + (p % 8) for p in range(32)],
            )
        s3b = s3[0:BC, :].rearrange("p (o n) -> p o n", n=N)
        sc = _squash(nc, mybir, small, BC, s3b)
        if last:
            v3f = small.tile([BC, O, N], f32, tag="v3f", name="v3f")
            nc.vector.tensor_mul(
                v3f[:], s3b,
                sc.unsqueeze(2).broadcast_to([BC, O, N]),
            )
            nc.scalar.dma_start(
                out=out_d[c * BC:(c + 1) * BC, :, :], in_=v3f[:]
            )
            return
        v3b = small.tile([32, ON], bf16, tag="v3b", name="v3b")
        nc.vector.memset(v3b[:], 0.0)
        nc.vector.tensor_mul(
            v3b[0:BC, :].rearrange("p (o n) -> p o n", n=N),
            s3b,
            sc.unsqueeze(2).broadcast_to([BC, O, N]),
        )
        vrep = small.tile([128, ON], bf16, tag="vrep", name="vrep")
        for q in range(4):
            nc.vector.stream_shuffle(
                vrep[q * 32:(q + 1) * 32, :],
                v3b[:],
                [2 * q + (j // 16) for j in range(32)],
            )

    # ---------------- running vsum + agreement tree ---------------------
    if it == 0:
        vcur = vrep
    elif it == 1:
        vs = small.tile([128, ON], bf16, tag=f"vs{c}", name=f"vs{c}", bufs=1)
        nc.vector.tensor_add(vs[:], vrep0s[c][:], vrep[:])
        vsums[c] = vs
        vcur = vs
    else:
        vs = vsums[c]
        nc.vector.tensor_add(vs[:], vs[:], vrep[:])
        vcur = vs
    bb = _tree(nc, mybir, tree, small, psW, U, vcur[:], c, it)

    # ---------------- softmax over o -> scatter into cbd ----------------
    c2 = small.tile([128, O, IB], bf16, tag="c2", name="c2", bufs=1)
    nc.scalar.activation(c2[:].transpose([0, 2, 1]), bb[:], AF.Exp)
    ssum = small.tile([128, IB], f32, tag="ssum", name="ssum")
    nc.vector.tensor_reduce(
        ssum[:], c2[:].transpose([0, 2, 1]), axis=AX.X, op=OP.add
    )
    rs = small.tile([128, IB], f32, tag="rs", name="rs")
    nc.vector.reciprocal(rs[:], ssum[:])
    c2n = small.tile([128, O, IB], bf16, tag="c2n", name="c2n")
    nc.gpsimd.tensor_mul(
        c2n[:], c2[:], rs[:].unsqueeze(1).broadcast_to([128, O, IB])
    )
    for b in range(BC):
        nc.sync.dma_start(
            out=cbd[b * 16:(b + 1) * 16, b:80:8, :],
            in_=c2n[b * 16:(b + 1) * 16, :, :],
        )


_CACHE = {}


def _get(n_routing: int, reps: int = 1):
    key = (n_routing, reps)
    if key not in _CACHE:
        _CACHE[key] = _build(n_routing, reps)
    return _CACHE[key]


def _bf16(a):
    import ml_dtypes

    return np.asarray(a, dtype=ml_dtypes.bfloat16)


def _prep_host(inputs: np.ndarray, W: np.ndarray):
    x = np.ascontiguousarray(np.asarray(inputs, dtype=np.float32))
    W = np.asarray(W, dtype=np.float32)
    # w2n[(il,v), ib, (o,n)] = W[ib*16+il, o, v, n]
    w2n = np.ascontiguousarray(
        W.reshape(IB, 16, O, V, N).transpose(1, 3, 0, 2, 4).reshape(128, IB, O * N)
    )
    return x, _bf16(w2n)


def _make_in_maps(inputs, W):
    x, w2n = _prep_host(inputs, W)
    in_maps = []
    for core in range(NCORES):
        xc = x[core * BLOC:(core + 1) * BLOC]              # [32, 1152, 8]
        # xbdh[c, il*8+v, ib, b*16+il] = xc[c*BC+b, ib*16+il, v]
        xr = xc.reshape(NCHUNK, BC, IB, 16, V)
        xbdh = np.zeros((NCHUNK, 128, IB, 128), dtype=np.float32)
        for il in range(16):
            xbdh[:, il * 8:(il + 1) * 8, :, il::16] = xr[:, :, :, il, :].transpose(
                0, 3, 2, 1
            )
        # xt2[(il,v), ib, b] = xc[b, ib*16+il, v]
        xt2 = np.ascontiguousarray(
            xc.reshape(BLOC, IB, 16, V).transpose(2, 3, 1, 0).reshape(128, IB, BLOC)
        )
        in_maps.append(
            {"xbdh": _bf16(xbdh), "w2": w2n, "xt2": _bf16(xt2)}
        )
    return in_maps


def kernel(inputs, W, n_routing):
    from concourse.bass_utils import run_bass_kernel_spmd

    n_routing = int(n_routing)
    nc = _get(n_routing)
    in_maps = _make_in_maps(inputs, W)
    res = run_bass_kernel_spmd(nc, in_maps, core_ids=list(range(NCORES)))
    outs = [res.results[i]["out"] for i in range(NCORES)]
    return np.concatenate(outs, axis=0).astype(np.float32)
